# revision 20
# baseline (speedup 1.0000x reference)
"""TRN2 Bass kernel for nn_Attention_20633022890922.

The reference module's einsum 'bqhk,bvhd->bqhd' contracts the attention-weight
head axis (k) and the value head axis (v) independently, so the product
factorizes into (sum_k softmax(...)) * (sum_v V) = 1 * Vsum.  The whole module
is therefore algebraically a single rank-64 linear layer:

    out = tokens @ Wv_sum @ Wo_sum + bo
      Wv_sum[h, d]  = sum_v Wv[h, v*64 + d]          (512 x 64)
      Wo_sum[d, e]  = sum_q Wo[q*64 + d, e]          (64 x 512)

(The only approximation is softmax summing to 1.0, ~1e-7 in fp32.)
Wq / Wk cancel entirely.

Device strategy: data-parallel over batch (8 batches -> 8 cores).  The
kernel is PE-bound at this size under a chip-level PE power budget
(only ~5 cores' worth of full-clock matmul activity is granted at a
time; dense per-core streams make cores queue for hot slots and the
slowest core runs much of its stream at half clock).  So:
  - X is quantized host-side to int8 (global per-core scale folded into
    the GEMM1 weights), 4.19 MB/core, upcast to fp16 inside the load DMA
    (SWDGE cast, exact).  Measured end-to-end error vs the fp32
    reference: max-rel 1.19e-2 / fro 1.19e-2 (budget 2e-2).
  - GEMM1 stationaries are duplicated to the full 128 array columns so
    the fast-weight-load path keeps LDWEIGHTS off the critical path;
    the duplicate T rows ride along for free (copy cost is free-dim
    driven) and GEMM2 reads rows 0-63.
  - PSUM->SBUF drains alternate between the only two engines with PSUM
    ports (DVE / ACT) at [128, 512] grain: this paces GEMM2 to ~63% PE
    duty (~84% per wave overall), which keeps all 8 cores inside the
    chip power budget simultaneously - measurably better on the slowest
    core than a maximally dense stream.
  - Y is written as Y^T fp16 [128, 2048] tiles (4 KB/partition runs),
    mostly on the sync HWDGE ring; the final wave's stores are split
    across both rings so the last completion receipt is short.  Host
    transposes/upcasts.
  - Wave 0 is loaded in two token halves so the first matmul starts
    ~2.5 us earlier.
"""

import time

import numpy as np

from concourse import bacc, mybir, tile
from concourse import bass_utils

B, N_TOK, HID, EMB, NH, HD = 8, 8192, 512, 512, 8, 64
N_CORES = 8
WAVE = 1024                   # tokens per load wave / compute group
NWAVE = N_TOK // WAVE         # 8

F32 = mybir.dt.float32
FP16 = mybir.dt.float16
I8 = mybir.dt.int8

_compiled = None


def _build():
    nc = bacc.Bacc(
        trn_type="TRN2", target_bir_lowering=False, debug=False, num_devices=N_CORES
    )

    # int8 tokens, wave-major: xq[w, p, j*1024 + t] = q(X[w*1024+t, j*128+p])
    xq_d = nc.dram_tensor("xq", [NWAVE, 128, 4096], I8, kind="ExternalInput")
    # GEMM1 weights (input scale folded, duplicated cols for full-width
    # stationary): cw[p, j*128 + c] = s_in*Wv_sum[j*128+p, c % 64]
    cw_d = nc.dram_tensor("cw", [128, 512], FP16, kind="ExternalInput")
    # GEMM2 weights: cwo[d, j2*128+h] = Wo_sum[d, j2*128+h]
    cwo_d = nc.dram_tensor("cwo", [64, 512], FP16, kind="ExternalInput")
    # output Y^T: y[j2, p, t] = Y[t, j2*128+p]  (fp16)
    y_d = nc.dram_tensor("y", [4, 128, N_TOK], FP16, kind="ExternalOutput")

    with tile.TileContext(nc) as tc:
        with (
            tc.tile_pool(name="const", bufs=1) as constp,
            tc.tile_pool(name="xt", bufs=NWAVE + 1) as xt_p,
            tc.tile_pool(name="tt", bufs=3) as tt_p,
            tc.tile_pool(name="yout", bufs=8) as y_p,
            # pt 2x[128,512] + py 4x[128,512] = 6 of 8 PSUM banks.
            # py at 512 grain paces GEMM2 to ~63% PE duty, which keeps all
            # 8 cores inside the chip-level full-clock power budget (dense
            # streams make cores queue for hot slots and the slowest core
            # runs most of its stream at half clock).
            tc.tile_pool(name="ps_t", bufs=2, space="PSUM") as ps_t,
            tc.tile_pool(name="ps_y", bufs=4, space="PSUM") as ps_y,
        ):
            cw = constp.tile([128, 512], FP16, tag="cw")
            cwo = constp.tile([64, 512], FP16, tag="cwo")
            nc.sync.dma_start(cw[:], cw_d[:])
            nc.scalar.dma_start(cwo[:], cwo_d[:])

            # PE warm-up: ~4.3us of dummy matmuls on a zeroed tile during
            # the otherwise-dead window between the engine preamble and the
            # first data arrival.  The HAM clock gate needs ~3.4us of
            # sustained PE activity before it grants full clock; without
            # this the first ~12 real matmuls run at half clock.  Sized to
            # end as the gate flips so almost no full-clock budget is spent.
            dum = constp.tile([128, 512], FP16, tag="dum")
            nc.vector.memset(dum[:], 0.0)

            # input loads up front: SWDGE cast-DMA int8 -> fp16.
            # wave 0 is split by token halves (all 4 hid blocks per half)
            # so the first GEMM1 group starts as soon as possible.
            x0 = []
            for h in range(2):
                t = xt_p.tile([128, 4, 512], FP16, tag="xt", name=f"xt0_{h}")
                src = xq_d[0].rearrange("p (j t) -> p j t", j=4)
                nc.gpsimd.dma_start(t[:], src[:, :, h * 512:(h + 1) * 512])
                x0.append(t)
            xt = [None]
            for w in range(1, NWAVE):
                t = xt_p.tile([128, 4096], FP16, tag="xt", name=f"xt{w}")
                nc.gpsimd.dma_start(t[:], xq_d[w])
                xt.append(t)

            dpy = ps_y.tile([128, 512], F32, tag="py", name="dummy_warm")
            for i in range(10):
                nc.tensor.matmul(
                    dpy[:], dum[:, 0:128], dum[:],
                    start=True, stop=True, skip_group_check=True,
                )

            ncopy = 0  # alternator for the two PSUM-drain engines

            def drain(dst, src):
                nonlocal ncopy
                if ncopy % 2 == 0:
                    nc.vector.tensor_copy(dst, src)
                else:
                    nc.scalar.copy(dst, src)
                ncopy += 1

            for w in range(NWAVE):
                # ---- GEMM1: pts[h] accumulates the 4 hid blocks j of
                # wave-half h.  Full-width dup stationary (FWL eligible).
                pts = [ps_t.tile([128, 512], F32, tag="pt", name=f"pt{w}_{h}")
                       for h in range(2)]
                if w == 0:
                    # half-major order so the first half's loads gate it
                    for h in range(2):
                        for j in range(4):
                            nc.tensor.matmul(
                                pts[h][:],
                                cw[:, j * 128:(j + 1) * 128],
                                x0[h][:, j, :],
                                start=(j == 0), stop=(j == 3),
                                skip_group_check=True,
                            )
                else:
                    # h-outer: pts[0]'s group completes after 4 matmuls so
                    # its drain overlaps the rest of GEMM1 (j-outer would
                    # hold both drains until the wave's 15th/16th matmul)
                    for h in range(2):
                        for j in range(4):
                            nc.tensor.matmul(
                                pts[h][:],
                                cw[:, j * 128:(j + 1) * 128],
                                xt[w][:, j * 1024 + h * 512:
                                      j * 1024 + (h + 1) * 512],
                                start=(j == 0), stop=(j == 3),
                                skip_group_check=True,
                            )

                tt = tt_p.tile([128, 1024], FP16, tag="tt")
                for h in range(2):
                    drain(tt[:, h * 512:(h + 1) * 512], pts[h][:])

                # ---- GEMM2 per output hid block j2 -> Y^T [128, 1024]
                for j2 in range(4):
                    if w % 2 == 0 and j2 == 0:
                        yb = {k: y_p.tile([128, 2048], FP16, tag="yb",
                                          name=f"yb{w}_{k}")
                              for k in range(4)}
                    off = (w % 2) * 1024
                    for h in range(2):
                        py = ps_y.tile([128, 512], F32, tag="py")
                        nc.tensor.matmul(
                            py[:],
                            cwo[:, j2 * 128:(j2 + 1) * 128],
                            tt[0:64, h * 512:(h + 1) * 512],
                            start=True, stop=True,
                        )
                        drain(yb[j2][:, off + h * 512:off + (h + 1) * 512],
                              py[:])

                    if w % 2 == 1:
                        dst = y_d[j2, :, (w - 1) * 1024:(w + 1) * 1024]
                        if w < NWAVE - 1:
                            # keep the ACT engine free for drains: regular
                            # stores all ride the sync HWDGE ring
                            nc.sync.dma_start(dst, yb[j2][:])
                        else:
                            # final stores: smaller, on both rings, so the
                            # last completion receipt is short and parallel
                            for half in range(2):
                                eng = (nc.sync if (j2 + half) % 2 == 0
                                       else nc.scalar)
                                eng.dma_start(
                                    dst[:, half * 1024:(half + 1) * 1024],
                                    yb[j2][:, half * 1024:(half + 1) * 1024],
                                )

    nc.compile()
    return nc


def _get_compiled():
    global _compiled
    if _compiled is None:
        _compiled = _build()
    return _compiled


def kernel(tokens, Wq, Wk, Wv, Wo, bo, _trace=False):
    tokens = np.asarray(tokens, dtype=np.float32)
    Wv = np.asarray(Wv, dtype=np.float32)
    Wo = np.asarray(Wo, dtype=np.float32)
    bo = np.asarray(bo, dtype=np.float32)

    wv_sum = Wv.reshape(HID, NH, HD).sum(axis=1)     # [512, 64]
    wo_sum = Wo.reshape(NH, HD, HID).sum(axis=0)     # [64, 512]
    cwo = np.ascontiguousarray(wo_sum.astype(np.float16))

    nc = _get_compiled()
    in_maps = []
    for b in range(N_CORES):
        X = tokens[b]
        s_in = np.abs(X).max() / 127.0
        xq = np.rint(X * (1.0 / s_in)).astype(np.int8)          # [8192, 512]
        xq = np.ascontiguousarray(
            xq.reshape(NWAVE, WAVE, 4, 128).transpose(0, 3, 2, 1)
            .reshape(NWAVE, 128, 4096)
        )
        # cw[p, j*128+c] = s_in * wv_sum[j*128+p, c%64]  (cols duplicated)
        w1 = (wv_sum * s_in).astype(np.float16).reshape(4, 128, 64)
        cw = np.ascontiguousarray(
            np.concatenate([w1, w1], axis=2)                     # [4, 128, 128]
            .transpose(1, 0, 2).reshape(128, 512)
        )
        in_maps.append({"xq": xq, "cw": cw, "cwo": cwo})

    # retry on transient device flakes (rare NRT_EXEC_UNIT wedges observed
    # under the axon PJRT path)
    for attempt in range(3):
        try:
            res = bass_utils.run_bass_kernel_spmd(
                nc, in_maps, core_ids=list(range(N_CORES)), trace=_trace
            )
            break
        except Exception:
            if attempt == 2:
                raise
            time.sleep(20)

    out = np.empty((N_CORES, N_TOK, HID), dtype=np.float32)
    for b in range(N_CORES):
        y = res.results[b]["y"]                                  # [4, 128, 8192] fp16
        out[b] = y.transpose(2, 0, 1).reshape(N_TOK, HID).astype(np.float32)
    if np.any(bo):
        out += bo
    if _trace:
        return out, res
    return out


if __name__ == "__main__":
    rng = np.random.default_rng(0)
    ins = {
        "tokens": rng.standard_normal((B, N_TOK, HID)).astype(np.float32),
        "Wq": (rng.standard_normal((HID, EMB)) * 0.02).astype(np.float32),
        "Wk": (rng.standard_normal((HID, EMB)) * 0.02).astype(np.float32),
        "Wv": (rng.standard_normal((HID, HID)) * 0.02).astype(np.float32),
        "Wo": (rng.standard_normal((EMB, HID)) * 0.02).astype(np.float32),
        "bo": np.zeros((HID,), dtype=np.float32),
    }
    out = kernel(**ins)
    print(out.shape, out.dtype)


# revision 21
# speedup vs baseline: 1.0582x; 1.0582x over previous
"""TRN2 Bass kernel for nn_Attention_20633022890922.

The reference module's einsum 'bqhk,bvhd->bqhd' contracts the attention-weight
head axis (k) and the value head axis (v) independently, so the product
factorizes into (sum_k softmax(...)) * (sum_v V) = 1 * Vsum.  The whole module
is therefore algebraically a single rank-64 linear layer:

    out = tokens @ Wv_sum @ Wo_sum + bo
      Wv_sum[h, d]  = sum_v Wv[h, v*64 + d]          (512 x 64)
      Wo_sum[d, e]  = sum_q Wo[q*64 + d, e]          (64 x 512)

(The only approximation is softmax summing to 1.0, ~1e-7 in fp32.)
Wq / Wk cancel entirely.

Device strategy: data-parallel over batch (8 batches -> 8 cores).  The
kernel is PE-bound at this size under a chip-level PE power budget
(only ~5 cores' worth of full-clock matmul activity is granted at a
time; dense per-core streams make cores queue for hot slots and the
slowest core runs much of its stream at half clock).  So:
  - X is quantized host-side to int8 (global per-core scale folded into
    the GEMM1 weights), 4.19 MB/core, upcast to fp16 inside the load DMA
    (SWDGE cast, exact).  Measured end-to-end error vs the fp32
    reference: max-rel 1.19e-2 / fro 1.19e-2 (budget 2e-2).
  - GEMM1 stationaries are duplicated to the full 128 array columns so
    the fast-weight-load path keeps LDWEIGHTS off the critical path;
    the duplicate T rows ride along for free (copy cost is free-dim
    driven) and GEMM2 reads rows 0-63.
  - PSUM->SBUF drains alternate between the only two engines with PSUM
    ports (DVE / ACT) at [128, 512] grain: this paces GEMM2 to ~63% PE
    duty (~84% per wave overall), which keeps all 8 cores inside the
    chip power budget simultaneously - measurably better on the slowest
    core than a maximally dense stream.
  - Y is written as Y^T fp16 [128, 2048] tiles (4 KB/partition runs),
    mostly on the sync HWDGE ring; the final wave's stores are split
    across both rings so the last completion receipt is short.  Host
    transposes/upcasts.
  - Wave 0 is loaded in two token halves so the first matmul starts
    ~2.5 us earlier.
"""

import time

import numpy as np

from concourse import bacc, mybir, tile
from concourse import bass_utils

B, N_TOK, HID, EMB, NH, HD = 8, 8192, 512, 512, 8, 64
N_CORES = 8
WAVE = 1024                   # tokens per load wave / compute group
NWAVE = N_TOK // WAVE         # 8

F32 = mybir.dt.float32
FP16 = mybir.dt.float16
I8 = mybir.dt.int8

_compiled = None


def _build():
    nc = bacc.Bacc(
        trn_type="TRN2", target_bir_lowering=False, debug=False, num_devices=N_CORES
    )

    # int8 tokens, wave-major: xq[w, p, j*1024 + t] = q(X[w*1024+t, j*128+p])
    xq_d = nc.dram_tensor("xq", [NWAVE, 128, 4096], I8, kind="ExternalInput")
    # GEMM1 weights (input scale folded, duplicated cols for full-width
    # stationary): cw[p, j*128 + c] = s_in*Wv_sum[j*128+p, c % 64]
    cw_d = nc.dram_tensor("cw", [128, 512], FP16, kind="ExternalInput")
    # GEMM2 weights: cwo[d, j2*128+h] = Wo_sum[d, j2*128+h]
    cwo_d = nc.dram_tensor("cwo", [64, 512], FP16, kind="ExternalInput")
    # output Y^T: y[j2, p, t] = Y[t, j2*128+p]  (fp16)
    y_d = nc.dram_tensor("y", [4, 128, N_TOK], FP16, kind="ExternalOutput")

    with tile.TileContext(nc) as tc:
        with (
            tc.tile_pool(name="const", bufs=1) as constp,
            tc.tile_pool(name="xt", bufs=NWAVE + 1) as xt_p,
            tc.tile_pool(name="tt", bufs=3) as tt_p,
            tc.tile_pool(name="yout", bufs=8) as y_p,
            # pt 2x[128,512] + py 4x[128,512] = 6 of 8 PSUM banks.
            # py at 512 grain paces GEMM2 to ~63% PE duty, which keeps all
            # 8 cores inside the chip-level full-clock power budget (dense
            # streams make cores queue for hot slots and the slowest core
            # runs most of its stream at half clock).
            tc.tile_pool(name="ps_t", bufs=2, space="PSUM") as ps_t,
            tc.tile_pool(name="ps_y", bufs=4, space="PSUM") as ps_y,
        ):
            cw = constp.tile([128, 512], FP16, tag="cw")
            cwo = constp.tile([64, 512], FP16, tag="cwo")
            nc.sync.dma_start(cw[:], cw_d[:])
            nc.scalar.dma_start(cwo[:], cwo_d[:])

            # PE warm-up: ~4.3us of dummy matmuls on a zeroed tile during
            # the otherwise-dead window between the engine preamble and the
            # first data arrival.  The HAM clock gate needs ~3.4us of
            # sustained PE activity before it grants full clock; without
            # this the first ~12 real matmuls run at half clock.  Sized to
            # end as the gate flips so almost no full-clock budget is spent.
            dum = constp.tile([128, 512], FP16, tag="dum")
            nc.vector.memset(dum[:], 0.0)

            # input loads up front: SWDGE cast-DMA int8 -> fp16.
            # wave 0 is split by token halves (all 4 hid blocks per half)
            # so the first GEMM1 group starts as soon as possible.
            x0 = []
            for h in range(2):
                t = xt_p.tile([128, 4, 512], FP16, tag="xt", name=f"xt0_{h}")
                src = xq_d[0].rearrange("p (j t) -> p j t", j=4)
                nc.gpsimd.dma_start(t[:], src[:, :, h * 512:(h + 1) * 512])
                x0.append(t)
            xt = [None]
            for w in range(1, NWAVE):
                t = xt_p.tile([128, 4096], FP16, tag="xt", name=f"xt{w}")
                nc.gpsimd.dma_start(t[:], xq_d[w])
                xt.append(t)

            dpy = ps_y.tile([128, 512], F32, tag="py", name="dummy_warm")
            for i in range(10):
                nc.tensor.matmul(
                    dpy[:], dum[:, 0:128], dum[:],
                    start=True, stop=True, skip_group_check=True,
                )

            ncopy = 0  # alternator for the two PSUM-drain engines

            def drain(dst, src):
                nonlocal ncopy
                if ncopy % 2 == 0:
                    nc.vector.tensor_copy(dst, src)
                else:
                    nc.scalar.copy(dst, src)
                ncopy += 1

            for w in range(NWAVE):
                # ---- GEMM1: pts[h] accumulates the 4 hid blocks j of
                # wave-half h.  Full-width dup stationary (FWL eligible).
                pts = [ps_t.tile([128, 512], F32, tag="pt", name=f"pt{w}_{h}")
                       for h in range(2)]
                if w == 0:
                    # half-major order so the first half's loads gate it
                    for h in range(2):
                        for j in range(4):
                            nc.tensor.matmul(
                                pts[h][:],
                                cw[:, j * 128:(j + 1) * 128],
                                x0[h][:, j, :],
                                start=(j == 0), stop=(j == 3),
                                skip_group_check=True,
                            )
                else:
                    # j-outer: holds GEMM1 at 4 stationary loads/wave and,
                    # with the drain-paced GEMM2, keeps per-core PE duty
                    # ~84% — low enough that the chip power arbiter admits
                    # all 8 cores to full clock together (h-outer/j-inner
                    # runs the stream faster but starves one core's clock
                    # grant and worsens the slowest core)
                    for j in range(4):
                        for h in range(2):
                            nc.tensor.matmul(
                                pts[h][:],
                                cw[:, j * 128:(j + 1) * 128],
                                xt[w][:, j * 1024 + h * 512:
                                      j * 1024 + (h + 1) * 512],
                                start=(j == 0), stop=(j == 3),
                                skip_group_check=True,
                            )

                tt = tt_p.tile([128, 1024], FP16, tag="tt")
                for h in range(2):
                    drain(tt[:, h * 512:(h + 1) * 512], pts[h][:])

                # ---- GEMM2 per output hid block j2 -> Y^T [128, 1024]
                for j2 in range(4):
                    if w % 2 == 0 and j2 == 0:
                        yb = {k: y_p.tile([128, 2048], FP16, tag="yb",
                                          name=f"yb{w}_{k}")
                              for k in range(4)}
                    off = (w % 2) * 1024
                    for h in range(2):
                        py = ps_y.tile([128, 512], F32, tag="py")
                        nc.tensor.matmul(
                            py[:],
                            cwo[:, j2 * 128:(j2 + 1) * 128],
                            tt[0:64, h * 512:(h + 1) * 512],
                            start=True, stop=True,
                        )
                        drain(yb[j2][:, off + h * 512:off + (h + 1) * 512],
                              py[:])

                    if w % 2 == 1:
                        dst = y_d[j2, :, (w - 1) * 1024:(w + 1) * 1024]
                        if w < NWAVE - 1:
                            # keep the ACT engine free for drains: regular
                            # stores all ride the sync HWDGE ring
                            nc.sync.dma_start(dst, yb[j2][:])
                        else:
                            # final stores: smaller, on both rings, so the
                            # last completion receipt is short and parallel
                            for half in range(2):
                                eng = (nc.sync if (j2 + half) % 2 == 0
                                       else nc.scalar)
                                eng.dma_start(
                                    dst[:, half * 1024:(half + 1) * 1024],
                                    yb[j2][:, half * 1024:(half + 1) * 1024],
                                )

    nc.compile()
    return nc


def _get_compiled():
    global _compiled
    if _compiled is None:
        _compiled = _build()
    return _compiled


def kernel(tokens, Wq, Wk, Wv, Wo, bo, _trace=False):
    tokens = np.asarray(tokens, dtype=np.float32)
    Wv = np.asarray(Wv, dtype=np.float32)
    Wo = np.asarray(Wo, dtype=np.float32)
    bo = np.asarray(bo, dtype=np.float32)

    wv_sum = Wv.reshape(HID, NH, HD).sum(axis=1)     # [512, 64]
    wo_sum = Wo.reshape(NH, HD, HID).sum(axis=0)     # [64, 512]
    cwo = np.ascontiguousarray(wo_sum.astype(np.float16))

    nc = _get_compiled()
    in_maps = []
    for b in range(N_CORES):
        X = tokens[b]
        s_in = np.abs(X).max() / 127.0
        xq = np.rint(X * (1.0 / s_in)).astype(np.int8)          # [8192, 512]
        xq = np.ascontiguousarray(
            xq.reshape(NWAVE, WAVE, 4, 128).transpose(0, 3, 2, 1)
            .reshape(NWAVE, 128, 4096)
        )
        # cw[p, j*128+c] = s_in * wv_sum[j*128+p, c%64]  (cols duplicated)
        w1 = (wv_sum * s_in).astype(np.float16).reshape(4, 128, 64)
        cw = np.ascontiguousarray(
            np.concatenate([w1, w1], axis=2)                     # [4, 128, 128]
            .transpose(1, 0, 2).reshape(128, 512)
        )
        in_maps.append({"xq": xq, "cw": cw, "cwo": cwo})

    # retry on transient device flakes (rare NRT_EXEC_UNIT wedges observed
    # under the axon PJRT path)
    for attempt in range(3):
        try:
            res = bass_utils.run_bass_kernel_spmd(
                nc, in_maps, core_ids=list(range(N_CORES)), trace=_trace
            )
            break
        except Exception:
            if attempt == 2:
                raise
            time.sleep(20)

    out = np.empty((N_CORES, N_TOK, HID), dtype=np.float32)
    for b in range(N_CORES):
        y = res.results[b]["y"]                                  # [4, 128, 8192] fp16
        out[b] = y.transpose(2, 0, 1).reshape(N_TOK, HID).astype(np.float32)
    if np.any(bo):
        out += bo
    if _trace:
        return out, res
    return out


if __name__ == "__main__":
    rng = np.random.default_rng(0)
    ins = {
        "tokens": rng.standard_normal((B, N_TOK, HID)).astype(np.float32),
        "Wq": (rng.standard_normal((HID, EMB)) * 0.02).astype(np.float32),
        "Wk": (rng.standard_normal((HID, EMB)) * 0.02).astype(np.float32),
        "Wv": (rng.standard_normal((HID, HID)) * 0.02).astype(np.float32),
        "Wo": (rng.standard_normal((EMB, HID)) * 0.02).astype(np.float32),
        "bo": np.zeros((HID,), dtype=np.float32),
    }
    out = kernel(**ins)
    print(out.shape, out.dtype)
